# revision 11
# baseline (speedup 1.0000x reference)
"""Trainium2 Bass kernel for nn_BaseMetricS2 (histogram_binning).

Math: the reference returns (mean(tp), mean(fp), mean(fn), mean(tn)) over the
(B, C) grid.  Summing the per-class identities over classes collapses the
whole problem to one weighted match-count per batch element:

    sum_c tp[b,c] = sum_px qw * [argmax_c pred == truth]      =: Wm_b
    sum_c fn[b,c] = sum_c fp[b,c] = S - Wm_b                  (S = sum qw)
    sum_c tn[b,c] = (C-2)*S + Wm_b

so no per-class histograms are needed on device.  Each of the 8 cores takes
one batch element (data-parallel over batch, per the sharding hint) and
computes unweighted per-(row-tile, partition) match counts; the host applies
the per-latitude quadrature weight (qw is constant along longitude) and the
final means.

Device pipeline per core, per [128-row x 720-col] chunk:
  1. DMA the 16 class planes into one SBUF tile [128, 16, 720].
  2. Stuff the class id into the 4 low mantissa bits of each f32 plane
     (tensor_scalar and/or, in place): v' = (v & ~0xF) | (15 - c).
  3. Grouped max-reduce over the class axis -> stuffed max per pixel.
  4. idx = (m' & 0xF) ^ 0xF  (argmax index; low-bit masking flips argmax only
     when the top-2 classes agree in their top 28 bits -- ~1e-6 of pixels,
     far below the output tolerance).
  5. tensor_tensor(is_equal(idx, truth)) -> f32 matched mask; ScalarE
     activation(Identity, accum_out) sums it per partition (TTR is broken on
     this runtime; the ACT-side sum also keeps the reduce off the busy DVE).

Row tiling: 721 rows = 5 full 128-row tiles + one final tile at rows
593..720 (overlapping rows 593..639, which the host masks out).
"""

import numpy as np

NLAT, NLON = 721, 1440
C = 16
N_CORES = 8
W_HALF = 720
TILE_R0 = (0, 128, 256, 384, 512, 593)
NCHUNK = len(TILE_R0) * 2  # 12

_CACHE = {}



def _register_fused_op():
    """Register STUFF_MAX_SEG, a custom DVE op used when fused=True:

        out[p, s, :] = running max over n of ((in0[p, s, n] | 0xFF) ^ in1[p, s, n])

    i.e. an inclusive max-scan along the innermost (class) axis that RESETS at
    each sub-dimension boundary, of the class-id-stuffed values.  The last
    element of each 16-class segment is then the stuffed max for that pixel.
    This fuses the whole stuffing pass into the reduce: one 1x pass over the
    16 planes instead of a 2x stuffing pass plus a 1x reduce pass.

    Segment reset is not expressible in the stock Spec language; we extend the
    scan lowering so that a registered reset-scan gets a SUB_DIM_DONE step
    state computing op(identity, expr) instead of op(CURR, expr).
    """
    from concourse import dve_ops, dve_spec
    from concourse.dve_spec import (
        Bin, Leaf, Scan, Spec, Src0, Src1, _has_src1 as has_src1, lower,
    )
    from concourse.dve_uop import AluOp, DveOpSpec, InpSel

    if "STUFF_MAX_SEG" in dve_ops._SUB_OPCODE_FOR_NAME:
        return next(o for o in dve_ops.OPS if o.name == "STUFF_MAX_SEG")

    stuffed = Bin(
        AluOp.BITWISE_XOR,
        Bin(AluOp.BITWISE_OR, Src0, Leaf(InpSel.MASK8_SL00)),
        Src1,
    )
    body = Scan(AluOp.MAX, stuffed)

    if not getattr(dve_spec, "_ant_reset_scan_patch", False):
        dve_spec._ant_reset_scan_patch = True
        dve_spec._ant_reset_scan_ids = set()
        orig = dve_spec._scan_overrides

        def _scan_overrides_with_reset(scans, node_stage):
            seed, step = orig(scans, node_stage)
            for scan in scans:
                if id(scan) in dve_spec._ant_reset_scan_ids:
                    d = node_stage[scan]
                    step[d] = dve_spec._Stage(scan.op, dve_spec.MaxNeg, scan.expr)
            return seed, step

        dve_spec._scan_overrides = _scan_overrides_with_reset
    dve_spec._ant_reset_scan_ids.add(id(body))

    def _ref(in0, in1, s0, s1, imm2):
        P = in0.shape[0]
        S = int(np.prod(in0.shape[1:-1]))
        N = in0.shape[-1]
        v = np.ascontiguousarray(in0).view(np.uint32).reshape(P, S, N)
        x = np.ascontiguousarray(np.broadcast_to(in1, in0.shape)).view(
            np.uint32
        ).reshape(P, S, N)
        st = ((v | np.uint32(0xFF)) ^ x).view(np.float32)
        return np.maximum.accumulate(st, axis=2).reshape(in0.shape)

    spec = Spec(body=body, reference=_ref)
    row = max(dve_ops._SUB_OPCODE_FOR_NAME.values()) + 1
    assert row < 0x20
    ver = "v3"  # TRN2
    sha = DveOpSpec(
        name="STUFF_MAX_SEG", opcode=row, uops=lower(spec, ver=ver),
        rd1_en=has_src1(spec),
    ).sha(ver)
    op = dve_ops.DveOp("STUFF_MAX_SEG", spec, subdim=True, uops_sha={ver: sha})
    dve_ops.OPS.append(op)
    dve_ops.CUSTOM_DVE_SPECS[op.name] = spec
    dve_ops._SUB_OPCODE_FOR_NAME[op.name] = row
    return op


def _build_program(repeat=1, pred_bufs=3, stuff_engine="vector", pairmax=False, fused=False):
    """Build the Bass program.  repeat>1 replays the whole body (same data)
    for slope-based wall-clock timing; the graded path uses repeat=1."""
    from contextlib import ExitStack

    import concourse.bacc as bacc
    import concourse.tile as tile
    from concourse import mybir

    F32 = mybir.dt.float32
    I32 = mybir.dt.int32
    Alu = mybir.AluOpType

    nc = bacc.Bacc("TRN2", target_bir_lowering=False, debug=False)
    pred = nc.dram_tensor("pred", [C, NLAT, NLON], F32, kind="ExternalInput").ap()
    truth = nc.dram_tensor("truth", [NLAT, NLON], I32, kind="ExternalInput").ap()
    out = nc.dram_tensor("out", [128, NCHUNK], F32, kind="ExternalOutput").ap()

    fused_op = _register_fused_op() if fused else None

    with tile.TileContext(nc) as tc, ExitStack() as ctx:
        pred_pool = ctx.enter_context(tc.tile_pool(name="pred", bufs=pred_bufs))
        tr_pool = ctx.enter_context(tc.tile_pool(name="tr", bufs=2))
        m_pool = ctx.enter_context(tc.tile_pool(name="m", bufs=2))
        idx_pool = ctx.enter_context(tc.tile_pool(name="idx", bufs=2))
        scr_pool = ctx.enter_context(tc.tile_pool(name="scr", bufs=2))
        acc_pool = ctx.enter_context(tc.tile_pool(name="acc", bufs=1))

        acc = acc_pool.tile([128, NCHUNK], F32)

        if fused:
            # class-id pattern for STUFF_MAX_SEG: plane c holds raw bits
            # 0xF0 | c.  Must be an f32-dtype tile holding those BIT PATTERNS
            # (denormals): int32-dtype operands are numerically converted to
            # f32 on DVE load, which would destroy the bit pattern.
            pat_pool = ctx.enter_context(tc.tile_pool(name="pat", bufs=1))
            pat = pat_pool.tile([128, C, W_HALF], F32)
            for c in range(C):
                bits = float(np.uint32(0xF0 | c).view(np.float32))
                nc.vector.memset(pat[:, c, :], bits)

        for _rep in range(repeat):
            for t, r0 in enumerate(TILE_R0):
                for h in range(2):
                    w0 = h * W_HALF
                    k = t * 2 + h

                    pt = pred_pool.tile([128, C, W_HALF], F32, tag="pred")
                    nc.sync.dma_start(
                        pt[:, :, :],
                        pred[:, r0 : r0 + 128, w0 : w0 + W_HALF].rearrange(
                            "c r w -> r c w"
                        ),
                    )
                    tt = tr_pool.tile([128, W_HALF], I32, tag="tr")
                    nc.sync.dma_start(tt[:, :], truth[r0 : r0 + 128, w0 : w0 + W_HALF])

                    if fused:
                        pt_r = pt[:, :, :].rearrange("p c w -> p w c")
                        nc.vector._custom_dve(
                            fused_op, out=pt_r, in0=pt_r,
                            in1=pat[:, :, :].rearrange("p c w -> p w c"),
                        )
                        m_ap = pt[:, C - 1, :]
                    else:
                        stuff_eng = getattr(nc, stuff_engine)
                        for c in range(C):
                            sl = pt[:, c, :].bitcast(I32)
                            stuff_eng.tensor_scalar(
                                sl, sl, -16, 15 - c, op0=Alu.bitwise_and, op1=Alu.bitwise_or
                            )

                        mt = m_pool.tile([128, W_HALF], F32, tag="m")
                        if pairmax:
                            for c in range(0, C, 2):
                                nc.gpsimd.tensor_tensor(
                                    pt[:, c, :], pt[:, c, :], pt[:, c + 1, :], op=Alu.max
                                )
                            red_in = pt[:, 0:C:2, :].rearrange("p c w -> p w c")
                        else:
                            red_in = pt[:, :, :].rearrange("p c w -> p w c")
                        nc.vector.tensor_reduce(
                            mt[:, :],
                            red_in,
                            axis=mybir.AxisListType.X,
                            op=Alu.max,
                        )
                        m_ap = mt[:, :]

                    it = idx_pool.tile([128, W_HALF], I32, tag="idx")
                    nc.vector.tensor_scalar(
                        it[:, :], m_ap.bitcast(I32), 15, 15,
                        op0=Alu.bitwise_and, op1=Alu.bitwise_xor,
                    )

                    st = scr_pool.tile([128, W_HALF], F32, tag="scr")
                    nc.vector.tensor_tensor(
                        st[:, :], it[:, :], tt[:, :], op=Alu.is_equal
                    )
                    nc.scalar.activation(
                        st[:, :], st[:, :], mybir.ActivationFunctionType.Identity,
                        accum_out=acc[:, k : k + 1],
                    )

        nc.sync.dma_start(out[:, :], acc[:, :])

    nc.compile()
    return nc


def _get_program():
    if "nc" not in _CACHE:
        _CACHE["nc"] = _build_program()
    return _CACHE["nc"]


def kernel(pred: np.ndarray, truth: np.ndarray, quad_weights: np.ndarray):
    from concourse.bass_utils import run_bass_kernel_spmd

    assert pred.shape == (N_CORES, C, NLAT, NLON), pred.shape
    pred = np.ascontiguousarray(pred, dtype=np.float32)
    truth_i32 = np.ascontiguousarray(truth.astype(np.int32))

    nc = _get_program()
    in_maps = [
        {"pred": pred[b], "truth": truth_i32[b]} for b in range(N_CORES)
    ]
    results = run_bass_kernel_spmd(nc, in_maps, list(range(N_CORES))).results

    # Host reduction: apply per-latitude quadrature weights and the means.
    qw = np.asarray(quad_weights, dtype=np.float64)
    w_row = qw[:, 0]  # qw is constant along longitude by construction
    S = float(qw.sum())

    wm = np.zeros(N_CORES, dtype=np.float64)
    for b in range(N_CORES):
        counts = np.asarray(results[b]["out"], dtype=np.float64)  # [128, 12]
        for t, r0 in enumerate(TILE_R0):
            per_row = counts[:, 2 * t] + counts[:, 2 * t + 1]  # [128]
            rows = r0 + np.arange(128)
            if t == len(TILE_R0) - 1:
                per_row = per_row[47:]  # rows 593..639 already counted in tile 4
                rows = rows[47:]
            wm[b] += float(np.dot(w_row[rows], per_row))

    denom = N_CORES * C
    tp_mean = wm.sum() / denom
    fp_mean = (N_CORES * S - wm.sum()) / denom
    fn_mean = fp_mean
    tn_mean = ((C - 2) * S * N_CORES + wm.sum()) / denom
    return (
        np.float32(tp_mean),
        np.float32(fp_mean),
        np.float32(fn_mean),
        np.float32(tn_mean),
    )


# revision 14
# speedup vs baseline: 14.3845x; 14.3845x over previous
"""Trainium2 Bass kernel for nn_BaseMetricS2 (histogram_binning).

Math: the reference returns (mean(tp), mean(fp), mean(fn), mean(tn)) over the
(B, C) grid.  Summing the per-class identities over classes collapses the
whole problem to one weighted match-count per batch element:

    sum_c tp[b,c] = sum_px qw * [argmax_c pred == truth]      =: Wm_b
    sum_c fn[b,c] = sum_c fp[b,c] = S - Wm_b                  (S = sum qw)
    sum_c tn[b,c] = (C-2)*S + Wm_b

so no per-class histograms are needed on device.  Each of the 8 cores takes
one batch element (data-parallel over batch, per the sharding hint) and
computes unweighted per-(row-tile, partition) match counts; the host applies
the per-latitude quadrature weight (qw is constant along longitude) and the
final means.

Device pipeline per core, per [128-row x 720-col] chunk:
  1. DMA the 16 class planes into one SBUF tile [128, 16, 720].
  2. Stuff the class id into the 4 low mantissa bits of each f32 plane
     (tensor_scalar and/or, in place): v' = (v & ~0xF) | (15 - c).
  3. Grouped max-reduce over the class axis -> stuffed max per pixel.
  4. idx = (m' & 0xF) ^ 0xF  (argmax index; low-bit masking flips argmax only
     when the top-2 classes agree in their top 28 bits -- ~1e-6 of pixels,
     far below the output tolerance).
  5. tensor_tensor(is_equal(idx, truth)) -> f32 matched mask; ScalarE
     activation(Identity, accum_out) sums it per partition (TTR is broken on
     this runtime; the ACT-side sum also keeps the reduce off the busy DVE).

Row tiling: 721 rows = 5 full 128-row tiles + one final tile at rows
593..720 (overlapping rows 593..639, which the host masks out).
"""

import numpy as np

NLAT, NLON = 721, 1440
C = 16
N_CORES = 8
W_HALF = 720
TILE_R0 = (0, 128, 256, 384, 512, 640)
NCHUNK = len(TILE_R0) * 2  # 12

_CACHE = {}



def _register_fused_op():
    """Register STUFF_MAX_SEG, a custom DVE op used when fused=True:

        out[p, s, :] = running max over n of ((in0[p, s, n] | 0xFF) ^ in1[p, s, n])

    i.e. an inclusive max-scan along the innermost (class) axis that RESETS at
    each sub-dimension boundary, of the class-id-stuffed values.  The last
    element of each 16-class segment is then the stuffed max for that pixel.
    This fuses the whole stuffing pass into the reduce: one 1x pass over the
    16 planes instead of a 2x stuffing pass plus a 1x reduce pass.

    Segment reset is not expressible in the stock Spec language; we extend the
    scan lowering so that a registered reset-scan gets a SUB_DIM_DONE step
    state computing op(identity, expr) instead of op(CURR, expr).
    """
    from concourse import dve_ops, dve_spec
    from concourse.dve_spec import (
        Bin, Leaf, Scan, Spec, Src0, Src1, _has_src1 as has_src1, lower,
    )
    from concourse.dve_uop import AluOp, DveOpSpec, InpSel

    if "STUFF_MAX_SEG" in dve_ops._SUB_OPCODE_FOR_NAME:
        return next(o for o in dve_ops.OPS if o.name == "STUFF_MAX_SEG")

    stuffed = Bin(
        AluOp.BITWISE_XOR,
        Bin(AluOp.BITWISE_OR, Src0, Leaf(InpSel.MASK8_SL00)),
        Src1,
    )
    body = Scan(AluOp.MAX, stuffed)

    if not getattr(dve_spec, "_ant_reset_scan_patch", False):
        dve_spec._ant_reset_scan_patch = True
        dve_spec._ant_reset_scan_ids = set()
        orig = dve_spec._scan_overrides

        def _scan_overrides_with_reset(scans, node_stage):
            seed, step = orig(scans, node_stage)
            for scan in scans:
                if id(scan) in dve_spec._ant_reset_scan_ids:
                    d = node_stage[scan]
                    step[d] = dve_spec._Stage(scan.op, dve_spec.MaxNeg, scan.expr)
            return seed, step

        dve_spec._scan_overrides = _scan_overrides_with_reset
    dve_spec._ant_reset_scan_ids.add(id(body))

    def _ref(in0, in1, s0, s1, imm2):
        P = in0.shape[0]
        S = int(np.prod(in0.shape[1:-1]))
        N = in0.shape[-1]
        v = np.ascontiguousarray(in0).view(np.uint32).reshape(P, S, N)
        x = np.ascontiguousarray(np.broadcast_to(in1, in0.shape)).view(
            np.uint32
        ).reshape(P, S, N)
        st = ((v | np.uint32(0xFF)) ^ x).view(np.float32)
        return np.maximum.accumulate(st, axis=2).reshape(in0.shape)

    spec = Spec(body=body, reference=_ref)
    row = max(dve_ops._SUB_OPCODE_FOR_NAME.values()) + 1
    assert row < 0x20
    ver = "v3"  # TRN2
    sha = DveOpSpec(
        name="STUFF_MAX_SEG", opcode=row, uops=lower(spec, ver=ver),
        rd1_en=has_src1(spec),
    ).sha(ver)
    op = dve_ops.DveOp("STUFF_MAX_SEG", spec, subdim=True, uops_sha={ver: sha})
    dve_ops.OPS.append(op)
    dve_ops.CUSTOM_DVE_SPECS[op.name] = spec
    dve_ops._SUB_OPCODE_FOR_NAME[op.name] = row
    return op


def _build_program(repeat=1, pred_bufs=4, stuff_engine="vector", pairmax=False, fused=False):
    """Build the Bass program.  repeat>1 replays the whole body (same data)
    for slope-based wall-clock timing; the graded path uses repeat=1."""
    import dataclasses
    from contextlib import ExitStack

    import concourse.bacc as bacc
    import concourse.tile as tile
    from concourse import mybir

    F32 = mybir.dt.float32
    I32 = mybir.dt.int32
    Alu = mybir.AluOpType

    nc = bacc.Bacc("TRN2", target_bir_lowering=False, debug=False)
    pred = nc.dram_tensor("pred", [C, NLAT, NLON], F32, kind="ExternalInput").ap()
    truth = nc.dram_tensor("truth", [NLAT, NLON], mybir.dt.uint8, kind="ExternalInput").ap()
    out = nc.dram_tensor("out", [128, NCHUNK], F32, kind="ExternalOutput").ap()

    fused_op = _register_fused_op() if fused else None

    with tile.TileContext(nc) as tc, ExitStack() as ctx:
        pred_pool = ctx.enter_context(tc.tile_pool(name="pred", bufs=pred_bufs))
        tr_pool = ctx.enter_context(tc.tile_pool(name="tr", bufs=2))
        m_pool = ctx.enter_context(tc.tile_pool(name="m", bufs=2))
        idx_pool = ctx.enter_context(tc.tile_pool(name="idx", bufs=2))
        scr_pool = ctx.enter_context(tc.tile_pool(name="scr", bufs=2))
        acc_pool = ctx.enter_context(tc.tile_pool(name="acc", bufs=1))

        acc = acc_pool.tile([128, NCHUNK], F32)

        if fused:
            # class-id pattern for STUFF_MAX_SEG: plane c holds raw bits
            # 0xF0 | c.  Must be an f32-dtype tile holding those BIT PATTERNS
            # (denormals): int32-dtype operands are numerically converted to
            # f32 on DVE load, which would destroy the bit pattern.
            pat_pool = ctx.enter_context(tc.tile_pool(name="pat", bufs=1))
            pat = pat_pool.tile([128, C], F32)
            for c in range(C):
                bits = float(np.uint32(0xF0 | c).view(np.float32))
                nc.vector.memset(pat[:, c : c + 1], bits)

        for _rep in range(repeat):
            for t, r0 in enumerate(TILE_R0):
                P = min(128, NLAT - r0)
                for h in range(2):
                    w0 = h * W_HALF
                    k = t * 2 + h

                    pt = pred_pool.tile([128, C, W_HALF], F32, tag="pred")
                    nc.sync.dma_start(
                        pt[:P, :, :],
                        pred[:, r0 : r0 + P, w0 : w0 + W_HALF].rearrange(
                            "c r w -> r c w"
                        ),
                    )
                    tt = tr_pool.tile([128, W_HALF], mybir.dt.uint8, tag="tr")
                    nc.sync.dma_start(tt[:P, :], truth[r0 : r0 + P, w0 : w0 + W_HALF])

                    if fused:
                        pt_r = pt[:P, :, :].rearrange("p c w -> p w c")
                        pb = pat[:P, :]
                        pat_b = dataclasses.replace(
                            pb, ap=[list(pb.ap[0]), [0, W_HALF], list(pb.ap[1])]
                        )
                        nc.vector._custom_dve(
                            fused_op, out=pt_r, in0=pt_r, in1=pat_b,
                        )
                        m_ap = pt[:P, C - 1, :]
                    else:
                        stuff_eng = getattr(nc, stuff_engine)
                        for c in range(C):
                            sl = pt[:, c, :].bitcast(I32)
                            stuff_eng.tensor_scalar(
                                sl, sl, -16, 15 - c, op0=Alu.bitwise_and, op1=Alu.bitwise_or
                            )

                        mt = m_pool.tile([128, W_HALF], F32, tag="m")
                        if pairmax:
                            for c in range(0, C, 2):
                                nc.gpsimd.tensor_tensor(
                                    pt[:, c, :], pt[:, c, :], pt[:, c + 1, :], op=Alu.max
                                )
                            red_in = pt[:, 0:C:2, :].rearrange("p c w -> p w c")
                        else:
                            red_in = pt[:, :, :].rearrange("p c w -> p w c")
                        nc.vector.tensor_reduce(
                            mt[:, :],
                            red_in,
                            axis=mybir.AxisListType.X,
                            op=Alu.max,
                        )
                        m_ap = mt[:, :]

                    it = idx_pool.tile([128, W_HALF], I32, tag="idx")
                    nc.vector.tensor_scalar(
                        it[:P, :], m_ap.bitcast(I32), 15, 15,
                        op0=Alu.bitwise_and, op1=Alu.bitwise_xor,
                    )

                    st = scr_pool.tile([128, W_HALF], F32, tag="scr")
                    nc.vector.tensor_tensor(
                        st[:P, :], it[:P, :], tt[:P, :], op=Alu.is_equal
                    )
                    nc.scalar.activation(
                        st[:P, :], st[:P, :], mybir.ActivationFunctionType.Identity,
                        accum_out=acc[:P, k : k + 1],
                    )

        nc.sync.dma_start(out[:, :], acc[:, :])

    nc.compile()
    return nc


def _get_program():
    if "nc" not in _CACHE:
        _CACHE["nc"] = _build_program(fused=True)
    return _CACHE["nc"]


def kernel(pred: np.ndarray, truth: np.ndarray, quad_weights: np.ndarray):
    from concourse.bass_utils import run_bass_kernel_spmd

    assert pred.shape == (N_CORES, C, NLAT, NLON), pred.shape
    pred = np.ascontiguousarray(pred, dtype=np.float32)
    truth_u8 = np.ascontiguousarray(truth.astype(np.uint8))

    nc = _get_program()
    in_maps = [
        {"pred": pred[b], "truth": truth_u8[b]} for b in range(N_CORES)
    ]
    results = run_bass_kernel_spmd(nc, in_maps, list(range(N_CORES))).results

    # Host reduction: apply per-latitude quadrature weights and the means.
    qw = np.asarray(quad_weights, dtype=np.float64)
    w_row = qw[:, 0]  # qw is constant along longitude by construction
    S = float(qw.sum())

    wm = np.zeros(N_CORES, dtype=np.float64)
    for b in range(N_CORES):
        counts = np.asarray(results[b]["out"], dtype=np.float64)  # [128, 12]
        for t, r0 in enumerate(TILE_R0):
            P = min(128, NLAT - r0)
            per_row = counts[:P, 2 * t] + counts[:P, 2 * t + 1]  # [P]
            rows = r0 + np.arange(P)
            wm[b] += float(np.dot(w_row[rows], per_row))

    denom = N_CORES * C
    tp_mean = wm.sum() / denom
    fp_mean = (N_CORES * S - wm.sum()) / denom
    fn_mean = fp_mean
    tn_mean = ((C - 2) * S * N_CORES + wm.sum()) / denom
    return (
        np.float32(tp_mean),
        np.float32(fp_mean),
        np.float32(fn_mean),
        np.float32(tn_mean),
    )
